# revision 17
# baseline (speedup 1.0000x reference)
"""2-layer GCN (GCNConv x2) on 8 trn2 NeuronCores.

Strategy (node/graph parallel, per sharding hint):
  - Nodes are ranked by in-degree (desc) and dealt round-robin to the 8
    cores in strata of 1024 ranks (128 nodes/core/stratum) so that every
    core's block b has a near-identical max in-degree -> uniform gather
    width k[b] across cores -> one SPMD program for all 8 cores.
  - norm(e) = dinv[src]*dinv[dst] factorizes: device stores h~ = dinv*h
    tables, gathers h~[src] per edge (indirect DMA, one instruction per
    128-node block), tree-reduces over the per-node slot dim on DVE,
    and applies the dinv[dst] factor once per output row.
  - Self-loops are plain edges (norm dinv^2 falls out of the same
    factorization). Slot padding points at a guaranteed-zero table row.
  - Layer 1 GEMM (x@W1) is computed replicated on every core (cheap)
    so no halo exchange is needed before aggregation 1; the only
    collective is an AllGather of the layer-2 table h~2.
"""

import numpy as np

N = 50000
E = 1000000
F_IN, F_HID, F_OUT = 64, 64, 32
P = 128
NCORES = 8
STR = P * NCORES          # 1024 ranks per stratum
NB = (N + STR - 1) // STR  # 49 blocks per core
NPAD = NB * STR            # 50176 padded node count
TAB = NPAD + P             # table rows; rows [NPAD, TAB) are zeros
ZROW = NPAD                # index of a guaranteed-zero row
LOCN = NB * P              # 6272 nodes per core

_last_results = None       # stash for test.py introspection
_nc_cache = {}             # kb-tuple -> compiled Bass program


def _host_prep(x, edge_index, W1, b1, W2, b2):
    src = np.asarray(edge_index[0], dtype=np.int64)
    dst = np.asarray(edge_index[1], dtype=np.int64)
    x = np.asarray(x, dtype=np.float32)

    deg = np.bincount(dst, minlength=N).astype(np.int64) + 1  # incl self-loop
    dinv = (1.0 / np.sqrt(deg.astype(np.float64))).astype(np.float32)

    # rank: sort by degree desc (stable) so same-block degrees are uniform
    node_perm = np.argsort(-deg, kind="stable")      # rank -> node
    rank = np.empty(N, dtype=np.int64)
    rank[node_perm] = np.arange(N)

    # rank -> (core, block, pos); local row on core = block*P + pos
    def decomp(r):
        i = r % STR
        return (i % NCORES), (r // STR), (i // NCORES)

    # real edges only; self-loops are handled by a direct strided DMA
    s_all = src
    d_all = dst
    r_s = rank[s_all]
    r_d = rank[d_all]
    c_d, b_d, p_d = decomp(r_d)
    slot = b_d * P + p_d                              # local row 0..LOCN-1
    c_s, b_s, p_s = decomp(r_s)
    ag_s = c_s * LOCN + b_s * P + p_s                 # allgather-order index

    # within-(core,slot) position j for each edge
    key = c_d * LOCN + slot
    order_e = np.argsort(key, kind="stable")
    ks = key[order_e]
    starts = np.searchsorted(ks, np.arange(NCORES * LOCN))
    cum = np.arange(len(ks), dtype=np.int64) - starts[ks]
    j = np.empty(len(ks), dtype=np.int64)
    j[order_e] = cum

    cnt = np.bincount(key, minlength=NCORES * LOCN)
    kb = cnt.reshape(NCORES, NB, P).max(axis=(0, 2)).astype(np.int64)
    kb = np.maximum(kb, 1)
    # S2 layout: [P, kb[b]] per block (edges only; self via direct DMA)
    off2 = np.zeros(NB + 1, dtype=np.int64)
    off2[1:] = np.cumsum(P * kb)
    TOT2 = int(off2[-1])
    # S1 layout: [P, kb[b]+1] per block, col 0 = self rank
    off1 = np.zeros(NB + 1, dtype=np.int64)
    off1[1:] = np.cumsum(P * (kb + 1))
    TOT1 = int(off1[-1])

    src1 = np.full((NCORES, TOT1), ZROW, dtype=np.int32)
    src2 = np.full((NCORES, TOT2), ZROW, dtype=np.int32)
    flat1 = off1[b_d] + p_d * (kb[b_d] + 1) + (j + 1)
    flat2 = off2[b_d] + p_d * kb[b_d] + j
    src1[c_d, flat1] = r_s.astype(np.int32)
    src2[c_d, flat2] = ag_s.astype(np.int32)
    # self rank for (core c, block b, partition p) = b*STR + p*NCORES + c
    bs, ps_ = np.meshgrid(np.arange(NB), np.arange(P), indexing="ij")
    for c in range(NCORES):
        selfr = (bs * STR + ps_ * NCORES + c).astype(np.int32)  # [NB, P]
        src1[c, off1[bs] + ps_ * (kb[bs] + 1)] = selfr

    # dinv in rank order, padded
    dinv_r = np.ones(NPAD, dtype=np.float32)
    dinv_r[:N][rank] = dinv                            # dinv_r[rank[v]] = dinv[v]
    dinv_A = dinv_r.reshape(NPAD // P, P).T.copy()     # [P, 392]
    dinv_B = dinv_r.reshape(NB, P, NCORES).transpose(2, 1, 0).copy()  # [c][P, NB]

    # x permuted+transposed, bf16
    import ml_dtypes
    xp = np.zeros((NPAD, F_IN), dtype=np.float32)
    xp[rank] = x                                       # xp[rank[v]] = x[v]
    xT = np.ascontiguousarray(xp.T).astype(ml_dtypes.bfloat16)  # [64, NPAD]

    W1b = np.asarray(W1, np.float32).astype(ml_dtypes.bfloat16)
    W2b = np.asarray(W2, np.float32).astype(ml_dtypes.bfloat16)
    b1_bc = np.ascontiguousarray(
        np.broadcast_to(np.asarray(b1, np.float32), (P, F_HID)))
    b2_bc = np.ascontiguousarray(
        np.broadcast_to(np.asarray(b2, np.float32), (P, F_OUT)))

    in_maps = []
    for c in range(NCORES):
        in_maps.append({
            "xT": xT, "W1": W1b, "W2": W2b, "B1": b1_bc, "B2": b2_bc,
            "DA": dinv_A, "DB": np.ascontiguousarray(dinv_B[c]),
            "S1": src1[c], "S2": src2[c],
        })
    return in_maps, [int(v) for v in kb], node_perm


def _reduce(nc, pool, G, k, F, dt):
    """Tree-sum G[P, k, F] (bf16) over axis 1 -> [P, 1, F] f32 tile."""
    cur, L = G, k
    first = True
    while L > 1:
        pairs, rem = L // 2, L % 2
        nxt = pool.tile([P, pairs + rem, F], dt.float32)
        nc.vector.tensor_add(nxt[:, :pairs], cur[:, :pairs], cur[:, pairs:2 * pairs])
        if rem:
            nc.vector.tensor_copy(nxt[:, pairs], cur[:, 2 * pairs])
        cur, L, first = nxt, pairs + rem, False
    if first:  # k == 1
        nxt = pool.tile([P, 1, F], dt.float32)
        nc.vector.tensor_copy(nxt[:], cur[:])
        cur = nxt
    return cur


def _build(kb):
    from contextlib import ExitStack
    import concourse.bass as bass
    import concourse.tile as tile
    from concourse import bacc, mybir
    from concourse.masks import make_identity

    dt = mybir.dt
    NBLK_A = NPAD // P
    TOT1 = P * (sum(kb) + NB)
    TOT2 = P * sum(kb)

    nc = bacc.Bacc("TRN2", target_bir_lowering=False, debug=False,
                   num_devices=NCORES)

    xT = nc.dram_tensor("xT", [F_IN, NPAD], dt.bfloat16, kind="ExternalInput").ap()
    W1 = nc.dram_tensor("W1", [F_IN, F_HID], dt.bfloat16, kind="ExternalInput").ap()
    W2 = nc.dram_tensor("W2", [F_HID, F_OUT], dt.bfloat16, kind="ExternalInput").ap()
    B1 = nc.dram_tensor("B1", [P, F_HID], dt.float32, kind="ExternalInput").ap()
    B2 = nc.dram_tensor("B2", [P, F_OUT], dt.float32, kind="ExternalInput").ap()
    DA = nc.dram_tensor("DA", [P, NBLK_A], dt.float32, kind="ExternalInput").ap()
    DB = nc.dram_tensor("DB", [P, NB], dt.float32, kind="ExternalInput").ap()
    S1 = nc.dram_tensor("S1", [TOT1], dt.int32, kind="ExternalInput").ap()
    S2 = nc.dram_tensor("S2", [TOT2], dt.int32, kind="ExternalInput").ap()
    OUT = nc.dram_tensor("OUT", [LOCN, F_OUT], dt.float32, kind="ExternalOutput").ap()
    H1 = nc.dram_tensor("H1", [TAB, F_HID], dt.bfloat16, kind="Internal").ap()
    H2P = nc.dram_tensor("H2P", [LOCN, F_OUT], dt.bfloat16, kind="Internal").ap()
    H2 = nc.dram_tensor("H2", [TAB, F_OUT], dt.bfloat16, kind="Internal").ap()

    AFT = mybir.ActivationFunctionType

    with ExitStack() as ctx:
        tc = ctx.enter_context(tile.TileContext(nc))
        const = ctx.enter_context(tc.tile_pool(name="const", bufs=1))
        w1s = const.tile([F_IN, F_HID], dt.bfloat16)
        nc.sync.dma_start(w1s[:], W1)
        w2s = const.tile([F_HID, F_OUT], dt.bfloat16)
        nc.sync.dma_start(w2s[:], W2)
        b1s = const.tile([P, F_HID], dt.float32)
        nc.sync.dma_start(b1s[:], B1)
        b2s = const.tile([P, F_OUT], dt.float32)
        nc.sync.dma_start(b2s[:], B2)
        das = const.tile([P, NBLK_A], dt.float32)
        nc.sync.dma_start(das[:], DA)
        dbs = const.tile([P, NB], dt.float32)
        nc.sync.dma_start(dbs[:], DB)
        ident = const.tile([P, P], dt.bfloat16)
        make_identity(nc, ident[:])
        zt = const.tile([P, F_HID], dt.bfloat16)
        nc.gpsimd.memset(zt[:], 0.0)
        nc.sync.dma_start(H1[NPAD:TAB, :], zt[:])
        nc.sync.dma_start(H2[NPAD:TAB, :], zt[:, :F_OUT])

        # ---- Phase A: h~1 = dinv * (x @ W1), replicated, 4 blocks/group ----
        UB = 8
        xpool = ctx.enter_context(tc.tile_pool(name="xp", bufs=4))
        hpool = ctx.enter_context(tc.tile_pool(name="hp", bufs=4))
        psA = ctx.enter_context(tc.tile_pool(name="psA", bufs=2, space="PSUM"))
        for jg in range(NBLK_A // UB):
            xt_t = xpool.tile([F_IN, UB * P], dt.bfloat16)
            nc.sync.dma_start(xt_t[:], xT[:, jg * UB * P:(jg + 1) * UB * P])
            ps4 = psA.tile([P, UB, F_HID], dt.float32, space="PSUM")
            hs4 = hpool.tile([P, UB, F_HID], dt.bfloat16)
            for u in range(UB):
                jblk = jg * UB + u
                nc.tensor.matmul(ps4[:, u, :], lhsT=xt_t[:, u * P:(u + 1) * P],
                                 rhs=w1s[:], start=True, stop=True)
                nc.vector.tensor_scalar_mul(hs4[:, u, :], ps4[:, u, :],
                                            das[:, jblk:jblk + 1])
            nc.scalar.dma_start(
                H1[jg * UB * P:(jg + 1) * UB * P, :].rearrange(
                    "(u p) f -> p u f", u=UB),
                hs4[:])

        # ---- Phase B+C: aggregate L1, relu, GEMM W2 -> h~2 part ----
        ipool = ctx.enter_context(tc.tile_pool(name="idx", bufs=4))
        gpool = ctx.enter_context(tc.tile_pool(name="g", bufs=4))
        rpool = ctx.enter_context(tc.tile_pool(name="r", bufs=3))
        opool = ctx.enter_context(tc.tile_pool(name="o", bufs=3))
        psB = ctx.enter_context(tc.tile_pool(name="psB", bufs=3, space="PSUM"))
        off = 0
        for b in range(NB):
            k = kb[b] + 1  # col 0 = self rank, cols 1..k = edges
            idx = ipool.tile([P, k], dt.int32)
            nc.scalar.dma_start(
                idx[:], S1[off:off + P * k].rearrange("(p k) -> p k", p=P))
            G = gpool.tile([P, k, F_HID], dt.bfloat16)
            for j in range(k):
                nc.gpsimd.indirect_dma_start(
                    out=G[:, j, :],
                    out_offset=None,
                    in_=H1,
                    in_offset=bass.IndirectOffsetOnAxis(ap=idx[:, j:j + 1], axis=0),
                )
            agg = _reduce(nc, rpool, G, k, F_HID, dt)
            t1 = opool.tile([P, F_HID], dt.float32)
            nc.scalar.activation(t1[:], agg[:, 0], AFT.Copy, scale=dbs[:, b:b + 1])
            t2 = opool.tile([P, F_HID], dt.float32)
            nc.vector.tensor_add(t2[:], t1[:], b1s[:])
            o1 = opool.tile([P, F_HID], dt.bfloat16)
            nc.vector.tensor_scalar_max(o1[:], t2[:], 0.0)
            pst = psB.tile([F_HID, P], dt.bfloat16, space="PSUM")
            nc.tensor.transpose(pst[:], o1[:], ident[:])
            o1T = opool.tile([F_HID, P], dt.bfloat16)
            nc.scalar.activation(o1T[:], pst[:], AFT.Copy)
            ps2 = psB.tile([P, F_OUT], dt.float32, space="PSUM")
            nc.tensor.matmul(ps2[:], lhsT=o1T[:], rhs=w2s[:], start=True, stop=True)
            h2s = hpool.tile([P, F_OUT], dt.bfloat16)
            nc.vector.tensor_scalar_mul(h2s[:], ps2[:], dbs[:, b:b + 1])
            nc.scalar.dma_start(H2P[b * P:(b + 1) * P, :], h2s[:])
            off += P * k

        # ---- AllGather h~2 ----
        nc.gpsimd.collective_compute(
            "AllGather", mybir.AluOpType.bypass,
            replica_groups=[list(range(NCORES))],
            ins=[H2P], outs=[H2[0:NPAD, :]],
        )

        # ---- Phase E: aggregate L2 -> output ----
        off = 0
        for b in range(NB):
            k = kb[b]
            idx = ipool.tile([P, k], dt.int32)
            nc.sync.dma_start(
                idx[:], S2[off:off + P * k].rearrange("(p k) -> p k", p=P))  # L2: sync is idle
            G = gpool.tile([P, k + 1, F_OUT], dt.bfloat16)
            # col 0 = self row from this core's own (pre-allgather) h~2 part
            nc.scalar.dma_start(G[:, 0, :], H2P[b * P:(b + 1) * P, :])
            for j in range(k):
                nc.gpsimd.indirect_dma_start(
                    out=G[:, j + 1, :],
                    out_offset=None,
                    in_=H2,
                    in_offset=bass.IndirectOffsetOnAxis(ap=idx[:, j:j + 1], axis=0),
                )
            agg = _reduce(nc, rpool, G, k + 1, F_OUT, dt)
            t1 = opool.tile([P, F_OUT], dt.float32)
            nc.scalar.activation(t1[:], agg[:, 0], AFT.Copy, scale=dbs[:, b:b + 1])
            o2 = opool.tile([P, F_OUT], dt.float32)
            nc.vector.tensor_add(o2[:], t1[:], b2s[:])
            nc.sync.dma_start(OUT[b * P:(b + 1) * P, :], o2[:])
            off += P * k

    nc.compile()
    return nc


def _ensure_ntff_hook():
    """Install the axon NTFF profile hook if the antenv stub lacks it."""
    import sys
    import types
    try:
        from antenv.axon_hooks import get_axon_ntff_profile_hook  # noqa: F401
        return
    except ImportError:
        pass
    try:
        import antenv
        from trn_agent_boot.trn_boot import _ntff_profile_via_ctypes
        hook = _ntff_profile_via_ctypes("/opt/axon/libaxon_pjrt.so")
        mod = types.ModuleType("antenv.axon_hooks")
        mod._hook = hook
        mod.get_axon_ntff_profile_hook = lambda: mod._hook
        mod.set_axon_ntff_profile_hook = lambda h: setattr(mod, "_hook", h)
        sys.modules["antenv.axon_hooks"] = mod
        antenv.axon_hooks = mod
    except Exception as e:  # tracing is best-effort
        print(f"ntff hook install failed: {e}")


def kernel(x, edge_index, W1, b1, W2, b2, _trace=False, _sim=False):
    global _last_results
    from concourse.bass_utils import run_bass_kernel_spmd
    if _trace:
        _ensure_ntff_hook()

    in_maps, kb, node_perm = _host_prep(x, edge_index, W1, b1, W2, b2)
    key = tuple(kb)
    nc = _nc_cache.get(key)
    if nc is None:
        nc = _nc_cache[key] = _build(kb)

    if _sim:
        from concourse.bass_interp import MultiCoreSim
        sim = MultiCoreSim(nc, num_cores=NCORES)
        cores = [sim.cores[i] for i in range(NCORES)]
        for c, core in enumerate(cores):
            for name, arr in in_maps[c].items():
                core.tensor(name)[:] = arr
        sim.simulate(check_with_hw=False)
        parts = [np.array(core.tensor("OUT")) for core in cores]
    else:
        res = run_bass_kernel_spmd(
            nc, in_maps, core_ids=list(range(NCORES)), trace=_trace)
        _last_results = res
        parts = [r["OUT"] for r in res.results]

    # unshard: core c, local row b*P+p -> rank b*STR + p*NCORES + c
    out = np.empty((N, F_OUT), dtype=np.float32)
    allp = np.stack(parts)                          # [c, LOCN, F_OUT]
    allp = allp.reshape(NCORES, NB, P, F_OUT)       # [c, b, p, f]
    by_rank = allp.transpose(1, 2, 0, 3).reshape(NPAD, F_OUT)  # rank-major
    out[node_perm] = by_rank[:N]
    return out
